# revision 1
# baseline (speedup 1.0000x reference)
"""AttentionBlock (GroupNorm + single-head self-attention + residual) on 8 TRN2 cores.

Strategy: pure data-parallel over batch (16 items -> 2 per core), no collectives.
All six big matmuls per item (Q, K, V, S=K^T Q, PV, proj) run in fp8-e4m3 with
perf_mode=DoubleRow (2 contraction sub-tiles per pass -> 2x PE throughput).
Weights are pre-scaled by 64 on the host (w ~ N(0, 1/c) would underflow fp8's
normal range); the 1/64 is folded into the PSUM evictions.  x is shipped as
bf16 (GroupNorm stats + residual tolerate it at rel-err ~6e-3 << the 2e-2 gate).

Per item (c=512 channels, n=1024 positions, 32 groups of 16 channels):
  - GroupNorm: bn_stats per channel-tile IN DMA-ARRIVAL ORDER, group-combine
    via block-diagonal selector matmul, rsqrt on DVE (fast-inverse-sqrt + 2
    Newton steps so the scalar engine's activation tables never swap) -> hn fp8.
  - Q,K: DoubleRow matmuls, both evicted on ACT (scale 1/64 + bias) -- the
    QK stretch has no other ACT work, and this frees DVE for the *other*
    item's GroupNorm, which is emitted interleaved so hn(1) is ready the
    moment item 1's QKV matmuls come up.
  - V computed TRANSPOSED: matmul(lhsT=hn, rhs=wv) -> vT [n, c], DVE evict.
  - S^T = K^T Q -> [j, i] tiles; eviction on ACT: e = exp(S*scale - 3) fp8.
    The -3 shift keeps e <= ~120 < fp8e4's 240 ceiling (fp8 downcast is
    NONSAT: overflow would be Inf); softmax cancels the shift exactly.
  - Denominators WITHOUT any elementwise pass over e: 16 accumulating
    ones(=1/4)-matmuls reduce e over j into a [1, n] PSUM row; they are
    emitted one round BEHIND the S tiles inside the interleave loops so the
    exp evictions are always ready (no PE wait) and no serial D-phase is
    left at the end.  A K=1 matmul broadcasts the row to 128 partitions and
    reciprocal_approx_fast gives recip = 4/D.
  - out = V @ e.  Item 0 evicts the PV PSUM on the otherwise-idle ACT as a
    pure 1/256 cast (exact, and independent of the D chain -- softmax
    normalization commutes with the projection) and applies recip at the
    proj eviction: o = psum*recip + (x + bpp) with x+bpp precomputed on ACT.
    Item 1 normalizes at the PV eviction (DVE, recip carries a 4x ou
    pre-scale) so its proj eviction is a scale+bias; the four tail
    evictions alternate ACT/DVE so the final drain runs two-wide.
HAM control: the PE's clock gate re-throttles to 1.2 GHz after ~3.4us idle.
During the serial GroupNorm(0) window the PE runs small warmup bursts that
are *gated on GroupNorm progress* (each burst's operand is poked by a tiny
DVE copy), so the PE shows activity in every HAM window without ever
racing ahead of the real work; QKV then starts at full 2.4 GHz.
Startup: all small constants ride in ONE packed DMA; x tiles and fp8
weights are laid out across the sync/scalar/gpsimd trigger queues in the
order compute consumes them.  Outputs fan out over three DMA queues.
"""

import numpy as np
import ml_dtypes

B_TOT, C, H, W = 16, 512, 32, 32
N = H * W            # 1024
NCORES = 8
BPC = B_TOT // NCORES  # 2 batch items per core
CT = C // 128        # 4 channel tiles
NT = N // 128        # 8 position tiles
NCH = N // 512       # 2 free-dim chunks of 512
GS = 16              # group size (channels per group)
EPS = 1e-5
SCALE = float(C) ** -0.5
WS = 64.0            # weight pre-scale (folded out at evictions)
OUS = 4.0            # recip pre-scale (ones=1/4 -> recip_sb = 4/D)
OCAST = 1.0 / 256    # item0 PV eviction cast scale (pure shift, exact)
EXPB = -3.0          # exp logit shift (cancels in softmax)
NVEC = 5             # gamma, beta, bq, bk, bpp
CB_W = NVEC * CT + 128  # const blob width (vectors + sel)

_CACHE = {}


def _build_bass():
    import concourse.bass as bass  # noqa: F401
    import concourse.tile as tile
    from concourse import bacc, mybir

    F32 = mybir.dt.float32
    BF16 = mybir.dt.bfloat16
    F8 = mybir.dt.float8e4
    Alu = mybir.AluOpType
    Act = mybir.ActivationFunctionType
    DR = mybir.MatmulPerfMode.DoubleRow

    nc = bacc.Bacc("TRN2", target_bir_lowering=False, debug=False,
                   num_devices=NCORES)

    x_ext = nc.dram_tensor("x", [BPC, 128, CT, N], BF16, kind="ExternalInput").ap()
    w_ext = {
        name: nc.dram_tensor(name, [128, CT, 512], F8, kind="ExternalInput").ap()
        for name in ("wq", "wk", "wv", "wp")
    }
    cb_ext = nc.dram_tensor("cb", [128, CB_W], F32, kind="ExternalInput").ap()
    out_ext = nc.dram_tensor("out", [BPC, 128, CT, N], F32, kind="ExternalOutput").ap()

    with tile.TileContext(nc) as tc:
        with (
            tc.tile_pool(name="consts", bufs=1) as consts,
            tc.tile_pool(name="xp", bufs=2) as xp,
            tc.tile_pool(name="hnp", bufs=2) as hnp,
            tc.tile_pool(name="qkp", bufs=2) as qkp,
            tc.tile_pool(name="vp", bufs=2) as vp,
            tc.tile_pool(name="ep", bufs=2) as ep,
            tc.tile_pool(name="oup", bufs=2) as oup,
            tc.tile_pool(name="outp", bufs=3) as outp,
            tc.tile_pool(name="rp", bufs=2) as rp,
            tc.tile_pool(name="xbp", bufs=1) as xbp,
            tc.tile_pool(name="smallp", bufs=8) as smallp,
            tc.tile_pool(name="psq", bufs=2, space="PSUM") as psq,
            tc.tile_pool(name="psv", bufs=2, space="PSUM") as psv,
            tc.tile_pool(name="pssm", bufs=2, space="PSUM") as pssm,
        ):
            # ---- DMA plan (per-queue issue order == consumption order) ----
            # sync:   x0t0a x0t2 wk | x1t0 x1t1 x1t2    out: ot0, ot3
            # scalar: x0t0b x0t1 wq | x1 none           out: ot1
            # gpsimd: x0t3 cb wv wp | x1t3              out: ot2
            # x0-t0 ships as two half-tiles on separate queues: it gates the
            # very first bn_stats (subtile deps let [0:512] start early).
            def xt_tile(b, t):
                return xp.tile([128, N], BF16, tag=f"x{t}", name=f"x_b{b}_t{t}")

            x0 = [xt_tile(0, t) for t in range(CT)]
            x1 = [xt_tile(1, t) for t in range(CT)]
            w_sb = {
                name: consts.tile([128, CT, 512], F8, tag=name, name=f"w_{name}")
                for name in ("wq", "wk", "wv", "wp")
            }
            cb_sb = consts.tile([128, CB_W], F32, tag="cb")

            nc.sync.dma_start(x0[0][:, 0:512], x_ext[0, :, 0, 0:512])
            nc.scalar.dma_start(x0[0][:, 512:1024], x_ext[0, :, 0, 512:1024])
            nc.gpsimd.dma_start(x0[3][:], x_ext[0, :, 3, :])
            nc.sync.dma_start(x0[2][:], x_ext[0, :, 2, :])
            nc.scalar.dma_start(x0[1][:], x_ext[0, :, 1, :])
            nc.gpsimd.dma_start(cb_sb[:], cb_ext[:])
            nc.scalar.dma_start(w_sb["wq"][:], w_ext["wq"][:])
            nc.sync.dma_start(w_sb["wk"][:], w_ext["wk"][:])
            nc.gpsimd.dma_start(w_sb["wv"][:], w_ext["wv"][:])
            nc.sync.dma_start(x1[0][:], x_ext[1, :, 0, :])
            nc.gpsimd.dma_start(w_sb["wp"][:], w_ext["wp"][:])
            nc.sync.dma_start(x1[1][:], x_ext[1, :, 1, :])
            nc.gpsimd.dma_start(x1[3][:], x_ext[1, :, 3, :])
            nc.sync.dma_start(x1[2][:], x_ext[1, :, 2, :])

            vec_sb = {
                name: cb_sb[:, i * CT:(i + 1) * CT]
                for i, name in enumerate(("gamma", "beta", "bq", "bk", "bpp"))
            }
            sel_sb = cb_sb[:, NVEC * CT:NVEC * CT + 128]
            ones_sb = consts.tile([128, 1], BF16, tag="ones")
            nc.vector.memset(ones_sb[:], 1.0 / OUS)
            onescol_sb = consts.tile([1, 128], BF16, tag="onescol")
            nc.vector.memset(onescol_sb[:], 1.0)
            magic_sb = consts.tile([128, 1], mybir.dt.int32, tag="magic")
            nc.vector.memset(magic_sb[:], 0x5F3759DF)
            expb_sb = consts.tile([128, 1], F32, tag="expb")
            nc.vector.memset(expb_sb[:], EXPB)

            # ---- HAM-warming machinery ----
            wu_sb = consts.tile([128, 512], BF16, tag="wu")
            nc.vector.memset(wu_sb[:], 0.0)
            ps_wu = psv.tile([128, 512], F32, tag="vmm", name="ps_warm")
            wu_state = {"started": False}

            def warm_burst(k, stop=False):
                for i in range(k):
                    nc.tensor.matmul(ps_wu[:], wu_sb[:, 0:128], wu_sb[:],
                                     start=not wu_state["started"],
                                     stop=stop and i == k - 1)
                    wu_state["started"] = True

            def warm_poke(src):
                # tiny DVE write into the warmup operand: the next warm_burst
                # waits for it, so the PE's activity tracks GroupNorm progress
                # instead of racing ahead and then idling into a re-throttle.
                nc.vector.tensor_copy(wu_sb[:, 508:510], src)

            warm_burst(16)

            def gn_stats(b, xts, mv, t):
                stats = smallp.tile([128, 2, 6], F32, tag="stats",
                                    name=f"st{b}_{t}")
                nc.vector.bn_stats(stats[:, 0, :], xts[t][:, 0:512])
                nc.vector.bn_stats(stats[:, 1, :], xts[t][:, 512:1024])
                nc.vector.bn_aggr(mv[:, t, :], stats[:])

            def gn_tail(b, xts, mv, warm=False):
                # s_all[:, 0, t]=mean_t, s_all[:, 1, t]=E[x^2]_t
                s_all = smallp.tile([128, 2, CT], F32, tag="s_all", name=f"s{b}")
                nc.vector.tensor_copy(s_all[:, 0, :], mv[:, :, 0])
                nc.vector.tensor_tensor(s_all[:, 1, :], mv[:, :, 0], mv[:, :, 0],
                                        Alu.mult)
                nc.vector.tensor_tensor(s_all[:, 1, :], s_all[:, 1, :],
                                        mv[:, :, 1], Alu.add)
                gs = pssm.tile([128, 2, CT], F32, tag="sm", name=f"gs{b}")
                nc.tensor.matmul(gs[:], sel_sb, s_all[:], start=True, stop=True)
                gsb = smallp.tile([128, 2, CT], F32, tag="gsb", name=f"gb{b}")
                nc.vector.tensor_copy(gsb[:], gs[:])
                ab = smallp.tile([128, 4, CT], F32, tag="ab", name=f"ab{b}")
                va = ab[:, 0, :]         # var
                vp_ = ab[:, 1, :]        # var + eps
                y = ab[:, 2, :]
                tmp = ab[:, 3, :]
                nc.vector.tensor_tensor(va, gsb[:, 0, :], gsb[:, 0, :], Alu.mult)
                nc.vector.tensor_tensor(va, gsb[:, 1, :], va, Alu.subtract)
                # rstd = rsqrt(var+eps) entirely on DVE (fast-inverse-sqrt seed
                # + 2 Newton steps) so the scalar engine's activation tables
                # never leave the exp set (table reloads are 2.7us each).
                nc.vector.tensor_scalar_add(vp_, va, EPS)
                I32 = mybir.dt.int32
                nc.vector.tensor_scalar(y.bitcast(I32), vp_.bitcast(I32), 1,
                                        None, Alu.arith_shift_right)
                nc.vector.tensor_tensor(y.bitcast(I32),
                                        magic_sb[:].to_broadcast([128, CT]),
                                        y.bitcast(I32), Alu.subtract)
                for _ in range(2):  # Newton: y *= 1.5 - 0.5*v*y^2
                    nc.vector.tensor_tensor(tmp, y, y, Alu.mult)
                    nc.vector.tensor_tensor(tmp, tmp, vp_, Alu.mult)
                    nc.vector.tensor_scalar(tmp, tmp, -0.5, 1.5, Alu.mult,
                                            Alu.add)
                    nc.vector.tensor_tensor(y, y, tmp, Alu.mult)
                if warm:
                    warm_poke(y[:, 0:2])
                    warm_burst(6)
                a_all = ab[:, 0, :]      # reuse var slot: a = rstd*gamma
                bsh = ab[:, 3, :]
                nc.vector.tensor_tensor(a_all, y, vec_sb["gamma"], Alu.mult)
                nc.vector.tensor_tensor(bsh, gsb[:, 0, :], a_all, Alu.mult)
                nc.vector.tensor_tensor(bsh, vec_sb["beta"], bsh, Alu.subtract)
                hn_sb = hnp.tile([128, CT, N], F8, tag="hn", name=f"hn{b}")
                for t in range(CT):
                    if t < 2:
                        nc.vector.tensor_scalar(hn_sb[:, t, :], xts[t][:],
                                                ab[:, 0, t:t + 1],
                                                ab[:, 3, t:t + 1],
                                                Alu.mult, Alu.add)
                    else:
                        nc.scalar.activation(hn_sb[:, t, :], xts[t][:],
                                             Act.Identity,
                                             bias=ab[:, 3, t:t + 1],
                                             scale=ab[:, 0, t:t + 1])
                    if warm and t == 1:
                        warm_poke(ab[:, 0, 0:2])
                        warm_burst(8, stop=True)
                return hn_sb

            def qk_tile(b, hn_sb, dst, wname, bname, t, on_act):
                # dst[:, t, :] = psum/WS + bias, psum = w^T @ hn (DoubleRow)
                ps = psq.tile([128, N], F32, tag="mm", name=f"ps_{wname}{b}_{t}")
                for itp in range(2):
                    lhs = w_sb[wname][:, 2 * itp:2 * itp + 2, t * 128:(t + 1) * 128]
                    for ch in range(NCH):
                        cs = slice(ch * 512, (ch + 1) * 512)
                        nc.tensor.matmul(ps[:, cs], lhs,
                                         hn_sb[:, 2 * itp:2 * itp + 2, cs],
                                         start=(itp == 0), stop=(itp == 1),
                                         perf_mode=DR)
                bias = vec_sb[bname][:, t:t + 1]
                if on_act:
                    nc.scalar.activation(dst[:, t, :], ps[:], Act.Identity,
                                         bias=bias, scale=1.0 / WS)
                else:
                    nc.vector.tensor_scalar(dst[:, t, :], ps[:], 1.0 / WS,
                                            bias, Alu.mult, Alu.add)

            def v_tile(b, hn_sb, vT_sb, jt):
                # vT[:, jt, :] = (hn^T @ wv)/WS  (DoubleRow, transposed out)
                ps = psv.tile([128, 512], F32, tag="vmm", name=f"psv{b}_{jt}")
                for itp in range(2):
                    nc.tensor.matmul(
                        ps[:], hn_sb[:, 2 * itp:2 * itp + 2, jt * 128:(jt + 1) * 128],
                        w_sb["wv"][:, 2 * itp:2 * itp + 2, :],
                        start=(itp == 0), stop=(itp == 1), perf_mode=DR)
                nc.vector.tensor_scalar(vT_sb[:, jt, :], ps[:], 1.0 / WS,
                                        None, Alu.mult)

            def s_tile(b, q_sb, k_sb, e_sb, jt):
                # e[:, jt, :] = exp(scale * k[:, :, jt-tile]^T @ q + EXPB)
                ps = psq.tile([128, N], F32, tag="mm", name=f"pss{b}_{jt}")
                for ctp in range(2):
                    lhs = k_sb[:, 2 * ctp:2 * ctp + 2, jt * 128:(jt + 1) * 128]
                    for ch in range(NCH):
                        cs = slice(ch * 512, (ch + 1) * 512)
                        nc.tensor.matmul(ps[:, cs], lhs,
                                         q_sb[:, 2 * ctp:2 * ctp + 2, cs],
                                         start=(ctp == 0), stop=(ctp == 1),
                                         perf_mode=DR)
                nc.scalar.activation(e_sb[:, jt, :], ps[:], Act.Exp,
                                     bias=expb_sb[:], scale=SCALE)

            def dsum_make(b):
                psd = [pssm.tile([1, 512], F32, tag="sm", name=f"d{b}_{ch}")
                       for ch in range(NCH)]
                return psd

            def dsum_acc(b, psd, e_sb, jts):
                # accumulate D rows for the given j-tiles (emitted one round
                # behind the S tiles so the exp evictions are always ready)
                for ch in range(NCH):
                    cs = slice(ch * 512, (ch + 1) * 512)
                    for jt in jts:
                        nc.tensor.matmul(psd[ch][:], ones_sb[:],
                                         e_sb[:, jt, cs],
                                         start=(jt == 0), stop=(jt == NT - 1))

            def dsum_tail(b, psd):
                drow = rp.tile([1, N], BF16, tag="drow", name=f"dr{b}")
                recip_sb = rp.tile([128, N], F32, tag="recip", name=f"rc{b}")
                for ch in range(NCH):
                    cs = slice(ch * 512, (ch + 1) * 512)
                    nc.scalar.copy(drow[:, cs], psd[ch][:])
                for ch in range(NCH):
                    cs = slice(ch * 512, (ch + 1) * 512)
                    bc = pssm.tile([128, 512], F32, tag="sm", name=f"bc{b}_{ch}")
                    nc.tensor.matmul(bc[:], onescol_sb[:], drow[:, cs],
                                     start=True, stop=True)
                    nc.vector.reciprocal_approx_fast(recip_sb[:, cs], bc[:])
                return recip_sb

            def pv_mms(b, vT_sb, e_sb, ct):
                ps = psq.tile([128, N], F32, tag="mm", name=f"pso{b}_{ct}")
                for jtp in range(4):
                    lhs = vT_sb[:, 2 * jtp:2 * jtp + 2, ct * 128:(ct + 1) * 128]
                    for ch in range(NCH):
                        cs = slice(ch * 512, (ch + 1) * 512)
                        nc.tensor.matmul(ps[:, cs], lhs,
                                         e_sb[:, 2 * jtp:2 * jtp + 2, cs],
                                         start=(jtp == 0), stop=(jtp == 3),
                                         perf_mode=DR)
                return ps

            out_engs = [nc.sync, nc.scalar, nc.gpsimd, nc.sync]

            def proj_tile(b, ou_sb, ot, evict):
                # evict: ('recip', recip_sb, xb_sb) -> o = ps*recip + xb
                #        ('act'|'dve', xts)        -> o = ps/(WS*OUS)+bpp, + x
                ps = psq.tile([128, N], F32, tag="mm", name=f"psp{b}_{ot}")
                for ctp in range(2):
                    lhs = w_sb["wp"][:, 2 * ctp:2 * ctp + 2, ot * 128:(ot + 1) * 128]
                    for ch in range(NCH):
                        cs = slice(ch * 512, (ch + 1) * 512)
                        nc.tensor.matmul(ps[:, cs], lhs,
                                         ou_sb[:, 2 * ctp:2 * ctp + 2, cs],
                                         start=(ctp == 0), stop=(ctp == 1),
                                         perf_mode=DR)
                o_sb = outp.tile([128, N], F32, tag="o", name=f"o{b}_{ot}")
                bias = vec_sb["bpp"][:, ot:ot + 1]
                if evict[0] == 'recip':
                    _, recip_sb, xb_sb = evict
                    nc.vector.tensor_tensor(o_sb[:], ps[:], recip_sb[:],
                                            Alu.mult)
                    nc.vector.tensor_tensor(o_sb[:], o_sb[:], xb_sb[:, ot, :],
                                            Alu.add)
                else:
                    kind, xts = evict
                    if kind == 'act':
                        nc.scalar.activation(o_sb[:], ps[:], Act.Identity,
                                             bias=bias, scale=1.0 / (WS * OUS))
                    else:
                        nc.vector.tensor_scalar(o_sb[:], ps[:],
                                                1.0 / (WS * OUS), bias,
                                                Alu.mult, Alu.add)
                    nc.vector.tensor_tensor(o_sb[:], o_sb[:], xts[ot][:],
                                            Alu.add)
                out_engs[ot].dma_start(out_ext[b, :, ot, :], o_sb[:])

            # ---- software pipeline over the two batch items ----
            # GroupNorm(0) with warmup bursts gated on its progress;
            # stats run in DMA-arrival order (t3 lands first on gpsimd).
            mv0 = smallp.tile([128, CT, 2], F32, tag="mv", name="mv0")
            for t in (0, 3, 2, 1):
                gn_stats(0, x0, mv0, t)
                warm_poke(mv0[:, t, :])
                warm_burst(6)
            h0 = gn_tail(0, x0, mv0, warm=True)

            # Q/K stretch for item 0 (ACT evictions) with item 1's GroupNorm
            # stats interleaved on the otherwise-idle DVE.
            q0 = qkp.tile([128, CT, N], F8, tag="q", name="q0")
            k0 = qkp.tile([128, CT, N], F8, tag="k", name="k0")
            mv1 = smallp.tile([128, CT, 2], F32, tag="mv", name="mv1")
            for t in range(CT):
                qk_tile(0, h0, k0, "wk", "bk", t, on_act=True)
                qk_tile(0, h0, q0, "wq", "bq", t, on_act=False)
                gn_stats(1, x1, mv1, t)
            h1 = gn_tail(1, x1, mv1)

            v0 = vp.tile([128, NT, 512], F8, tag="vT", name="vT0")
            for jt in range(NT):
                v_tile(0, h0, v0, jt)

            # item0 S-phase interleaved with item1 QKV + item0 D-sums (lagged)
            e0 = ep.tile([128, NT, N], F8, tag="e", name="e0")
            q1 = qkp.tile([128, CT, N], F8, tag="q", name="q1")
            k1 = qkp.tile([128, CT, N], F8, tag="k", name="k1")
            v1 = vp.tile([128, NT, 512], F8, tag="vT", name="vT1")
            psd0 = dsum_make(0)
            for r in range(CT):
                s_tile(0, q0, k0, e0, 2 * r)
                s_tile(0, q0, k0, e0, 2 * r + 1)
                qk_tile(1, h1, k1, "wk", "bk", r, on_act=True)
                qk_tile(1, h1, q1, "wq", "bq", r, on_act=False)
                v_tile(1, h1, v1, 2 * r)
                v_tile(1, h1, v1, 2 * r + 1)
                if r > 0:
                    dsum_acc(0, psd0, e0, (2 * r - 2, 2 * r - 1))
            dsum_acc(0, psd0, e0, (NT - 2, NT - 1))
            # pv0: PSUM evicted on ACT as a pure 1/256 cast (no recip
            # dependency); normalization happens at proj0's eviction.
            ou0 = oup.tile([128, CT, N], F8, tag="ou", name="ou0")
            for ct in range(CT):
                ps = pv_mms(0, v0, e0, ct)
                nc.scalar.mul(ou0[:, ct, :], ps[:], OCAST)
            r0 = dsum_tail(0, psd0)
            xb0 = xbp.tile([128, CT, N], BF16, tag="xb", name="xb0")
            for t in range(CT):
                nc.scalar.activation(xb0[:, t, :], x0[t][:], Act.Identity,
                                     bias=vec_sb["bpp"][:, t:t + 1])

            # item0 proj interleaved with item1 S-phase + item1 D-sums
            e1 = ep.tile([128, NT, N], F8, tag="e", name="e1")
            psd1 = dsum_make(1)
            for r in range(CT):
                proj_tile(0, ou0, r, ('recip', r0, xb0))
                s_tile(1, q1, k1, e1, 2 * r)
                s_tile(1, q1, k1, e1, 2 * r + 1)
                if r > 0:
                    dsum_acc(1, psd1, e1, (2 * r - 2, 2 * r - 1))
            # pv1-ct0's matmuls cover the wait for the last exp1 eviction
            ou1 = oup.tile([128, CT, N], F8, tag="ou", name="ou1")
            dsum_acc(1, psd1, e1, (NT - 2,))
            ps10 = pv_mms(1, v1, e1, 0)
            dsum_acc(1, psd1, e1, (NT - 1,))
            r1 = dsum_tail(1, psd1)
            nc.vector.tensor_tensor(ou1[:, 0, :], ps10[:], r1[:], Alu.mult)
            for ct in range(1, CT):
                ps = pv_mms(1, v1, e1, ct)
                nc.vector.tensor_tensor(ou1[:, ct, :], ps[:], r1[:], Alu.mult)
            for r in range(CT):
                proj_tile(1, ou1, r, ('act' if r % 2 == 0 else 'dve', x1))

    nc.compile()
    return nc


def _prep_vec(v):
    # [C] f32 -> [128, CT] with v_sb[p, t] = v[t*128 + p]
    return np.ascontiguousarray(
        np.asarray(v, dtype=np.float32).reshape(CT, 128).T)


def _prep_w(w):
    # [C, C] (out, in) -> lhsT layout [128, CT, 512] fp8e4, pre-scaled by WS:
    # w_sb[p, it, o] = w[o, it*128 + p] * WS
    wT = np.asarray(w, dtype=np.float32).T * WS
    arr = wT.reshape(CT, 128, C).transpose(1, 0, 2)
    return np.clip(np.ascontiguousarray(arr), -240.0, 240.0).astype(
        ml_dtypes.float8_e4m3)


def kernel(x, gamma, beta, wq, bq, wk, bk, wv, bv, wp, bp):
    from concourse.bass_utils import run_bass_kernel_spmd

    nc = _CACHE.get("nc")
    if nc is None:
        nc = _CACHE["nc"] = _build_bass()

    x = np.asarray(x, dtype=np.float32)
    # [16, C, H, W] -> [16, 128, CT, N] bf16
    xr = np.ascontiguousarray(
        x.reshape(B_TOT, CT, 128, N).transpose(0, 2, 1, 3)).astype(
        ml_dtypes.bfloat16)

    bpp = np.asarray(wp, np.float32) @ np.asarray(bv, np.float32) \
        + np.asarray(bp, np.float32)
    sel = np.kron(np.eye(128 // GS, dtype=np.float32),
                  np.full((GS, GS), 1.0 / GS, dtype=np.float32))
    cb = np.empty((128, CB_W), dtype=np.float32)
    for i, v in enumerate((gamma, beta, bq, bk, bpp)):
        cb[:, i * CT:(i + 1) * CT] = _prep_vec(v)
    cb[:, NVEC * CT:] = sel
    common = {
        "wq": _prep_w(wq), "wk": _prep_w(wk), "wv": _prep_w(wv),
        "wp": _prep_w(wp), "cb": cb,
    }
    in_maps = [
        {"x": np.ascontiguousarray(xr[c * BPC:(c + 1) * BPC]), **common}
        for c in range(NCORES)
    ]
    res = run_bass_kernel_spmd(nc, in_maps, core_ids=list(range(NCORES)))
    # [BPC, 128, CT, N] per core -> [16, C, H, W]
    out = np.concatenate([r["out"] for r in res.results], axis=0)
    return np.ascontiguousarray(
        out.transpose(0, 2, 1, 3)).reshape(B_TOT, C, H, W)



# revision 8
# speedup vs baseline: 1.1287x; 1.1287x over previous
"""AttentionBlock (GroupNorm + single-head self-attention + residual) on 8 TRN2 cores.

Strategy: pure data-parallel over batch (16 items -> 2 per core), no collectives.
All six big matmuls per item (Q, K, V, S=K^T Q, PV, proj) run in fp8-e4m3 with
perf_mode=DoubleRow (2 contraction sub-tiles per pass -> 2x PE throughput).
Weights are pre-scaled by 64 on the host (w ~ N(0, 1/c) would underflow fp8's
normal range); the 1/64 is folded into the PSUM evictions.  x is shipped as
bf16; the output returns as bf16 and is upcast on the host (rel-err ~6e-3 <<
the 2e-2 gate).

Per item (c=512 channels, n=1024 positions, 32 groups of 16 channels):
  - GroupNorm stats are split across engines: tiles 0,1 via bn_stats on DVE,
    tiles 2,3 via activation-accum (Identity/Square row-sums) on ACT -- the
    Identity pass doubles as the xb = x + bpp precompute.  Group-combine via
    block-diagonal selector matmul per PAIR of tiles, rsqrt on DVE
    (fast-inverse-sqrt + 2 Newton steps; ACT's exp table never swaps).
    hn tiles evict as their pair's tail completes, so QKV starts ~9us earlier
    than an all-tiles-then-evict scheme.
  - First DR pass of K0/Q0/V(j0,j1) runs on hn tiles 0,1 only (DoubleRow
    contracts channel pairs), overlapping the tail of the stats pipeline.
  - All big-matmul PSUM tiles are single-bank [128,512] chunks, tag-rotated
    4 deep, so evictions pipeline tighter than [128,1024] x 2.
  - S^T = K^T Q -> [j, i] tiles; eviction on ACT: e = exp(S*scale - 3) fp8.
    The -3 shift keeps e <= ~120 < fp8e4's 240 ceiling; softmax cancels it.
  - Denominators: DoubleRow ones(=1/4)-matmuls reduce e over j-tile PAIRS into
    [1, n] PSUM rows, emitted one round behind the S tiles.  drow copies on
    DVE, K=1 broadcast matmul, reciprocal_approx_fast -> recip = 4/D.
  - Both items: PV eviction applies recip on DVE (ou = psum * recip, a 4x
    pre-scaled fp8 attention output); proj eviction is ONE fused DVE op
    o = (psum/(64*4)) + xb via scalar_tensor_tensor, written bf16.
  - Item1's last two V j-tiles are deferred past its S phase so the PE has
    work while the exp8 -> dsum -> drow -> bcast -> recip chain completes.
HAM control: warmup bursts (gated on GroupNorm progress via tiny DVE pokes
into the burst operand) keep the PE clock at 2.4 GHz through the serial head.
Startup: input DMA triggers ride sync/gpsimd/tensor queues so the scalar
engine is free for stats from the first microsecond; x tiles land in the
order stats consume them.  Outputs (bf16) fan out over sync/gpsimd queues.
"""

import numpy as np
import ml_dtypes

B_TOT, C, H, W = 16, 512, 32, 32
N = H * W            # 1024
NCORES = 8
BPC = B_TOT // NCORES  # 2 batch items per core
CT = C // 128        # 4 channel tiles
NT = N // 128        # 8 position tiles
NCH = N // 512       # 2 free-dim chunks of 512
GS = 16              # group size (channels per group)
EPS = 1e-5
SCALE = float(C) ** -0.5
WS = 64.0            # weight pre-scale (folded out at evictions)
OUS = 4.0            # recip pre-scale (ones=1/4 -> recip_sb = 4/D)
EXPB = -3.0          # exp logit shift (cancels in softmax)
NVEC = 5             # gamma, beta, bq, bk, bpp
CB_W = NVEC * CT + 128  # const blob width (vectors + sel)

_CACHE = {}


def _build_bass():
    import concourse.bass as bass  # noqa: F401
    import concourse.tile as tile
    from concourse import bacc, mybir

    F32 = mybir.dt.float32
    BF16 = mybir.dt.bfloat16
    F8 = mybir.dt.float8e4
    Alu = mybir.AluOpType
    Act = mybir.ActivationFunctionType
    DR = mybir.MatmulPerfMode.DoubleRow

    nc = bacc.Bacc("TRN2", target_bir_lowering=False, debug=False,
                   num_devices=NCORES)

    x_ext = nc.dram_tensor("x", [BPC, 128, CT, N], BF16, kind="ExternalInput").ap()
    w_ext = {
        name: nc.dram_tensor(name, [128, CT, 512], F8, kind="ExternalInput").ap()
        for name in ("wq", "wk", "wv", "wp")
    }
    cb_ext = nc.dram_tensor("cb", [128, CB_W], F32, kind="ExternalInput").ap()
    out_ext = nc.dram_tensor("out", [BPC, 128, CT, N], BF16, kind="ExternalOutput").ap()

    with tile.TileContext(nc) as tc:
        with (
            tc.tile_pool(name="consts", bufs=1) as consts,
            tc.tile_pool(name="xp", bufs=2) as xp,
            tc.tile_pool(name="hnp", bufs=2) as hnp,
            tc.tile_pool(name="qkp", bufs=2) as qkp,
            tc.tile_pool(name="vp", bufs=2) as vp,
            tc.tile_pool(name="ep", bufs=2) as ep,
            tc.tile_pool(name="oup", bufs=2) as oup,
            tc.tile_pool(name="outp", bufs=3) as outp,
            tc.tile_pool(name="rp", bufs=2) as rp,
            tc.tile_pool(name="xbp", bufs=2) as xbp,
            tc.tile_pool(name="scrp", bufs=2) as scrp,
            tc.tile_pool(name="smallp", bufs=8) as smallp,
            tc.tile_pool(name="psq", bufs=4, space="PSUM") as psq,
            tc.tile_pool(name="psv", bufs=2, space="PSUM") as psv,
            tc.tile_pool(name="pssm", bufs=2, space="PSUM") as pssm,
        ):
            # ---- DMA plan: scalar engine issues NO input triggers (it runs
            # the ACT-side stats from ~7us); weights ride the tensor queue.
            def xt_tile(b, t):
                return xp.tile([128, N], BF16, tag=f"x{t}", name=f"x_b{b}_t{t}")

            x0 = [xt_tile(0, t) for t in range(CT)]
            x1 = [xt_tile(1, t) for t in range(CT)]
            w_sb = {
                name: consts.tile([128, CT, 512], F8, tag=name, name=f"w_{name}")
                for name in ("wq", "wk", "wv", "wp")
            }
            cb_sb = consts.tile([128, CB_W], F32, tag="cb")

            nc.sync.dma_start(x0[0][:, 0:512], x_ext[0, :, 0, 0:512])
            nc.gpsimd.dma_start(cb_sb[:], cb_ext[:])
            nc.scalar.dma_start(w_sb["wk"][:], w_ext["wk"][:])
            nc.sync.dma_start(x0[1][:], x_ext[0, :, 1, :])
            nc.gpsimd.dma_start(x0[0][:, 512:1024], x_ext[0, :, 0, 512:1024])
            nc.scalar.dma_start(x0[2][:], x_ext[0, :, 2, :])
            nc.scalar.dma_start(x0[3][:], x_ext[0, :, 3, :])
            nc.gpsimd.dma_start(w_sb["wv"][:], w_ext["wv"][:])
            nc.scalar.dma_start(w_sb["wq"][:], w_ext["wq"][:])
            nc.sync.dma_start(x1[0][:], x_ext[1, :, 0, :])
            nc.gpsimd.dma_start(w_sb["wp"][:], w_ext["wp"][:])
            nc.sync.dma_start(x1[1][:], x_ext[1, :, 1, :])
            nc.gpsimd.dma_start(x1[2][:], x_ext[1, :, 2, :])
            nc.gpsimd.dma_start(x1[3][:], x_ext[1, :, 3, :])

            vec_sb = {
                name: cb_sb[:, i * CT:(i + 1) * CT]
                for i, name in enumerate(("gamma", "beta", "bq", "bk", "bpp"))
            }
            sel_sb = cb_sb[:, NVEC * CT:NVEC * CT + 128]
            # [128, 2, 16]: DR ldweights wants the plane stride 16B-aligned
            ones_sb = consts.tile([128, 2, 16], F8, tag="ones")
            nc.vector.memset(ones_sb[:], 1.0 / OUS)
            onescol_sb = consts.tile([1, 128], BF16, tag="onescol")
            nc.vector.memset(onescol_sb[:], 1.0)
            magic_sb = consts.tile([128, 1], mybir.dt.int32, tag="magic")
            nc.vector.memset(magic_sb[:], 0x5F3759DF)
            expb_sb = consts.tile([128, 1], F32, tag="expb")
            nc.vector.memset(expb_sb[:], EXPB)

            # ---- HAM-warming machinery ----
            wu_sb = consts.tile([128, 512], BF16, tag="wu")
            nc.vector.memset(wu_sb[:], 0.0)
            ps_wu = psv.tile([128, 512], F32, tag="vmm", name="ps_warm")
            wu_state = {"started": False}

            def warm_burst(k, stop=False):
                for i in range(k):
                    nc.tensor.matmul(ps_wu[:], wu_sb[:, 0:128], wu_sb[:],
                                     start=not wu_state["started"],
                                     stop=stop and i == k - 1)
                    wu_state["started"] = True

            def warm_poke(src):
                nc.vector.tensor_copy(wu_sb[:, 508:510], src)

            I32 = mybir.dt.int32

            def gn_stats_dve(b, xts, mv, t):
                # DVE bn_stats path -> mv[:, t, :] = (mean, var)
                stats = smallp.tile([128, 2, 6], F32, tag="stats",
                                    name=f"st{b}_{t}")
                nc.vector.bn_stats(stats[:, 0, :], xts[t][:, 0:512])
                nc.vector.bn_stats(stats[:, 1, :], xts[t][:, 512:1024])
                nc.vector.bn_aggr(mv[:, t, :], stats[:])

            def gn_mv_to_sall(b, mv, s_all, t0):
                # s_all[:,0,t]=mean_t, s_all[:,1,t]=E[x^2]_t for tiles t0,t0+1
                sl = slice(t0, t0 + 2)
                nc.vector.tensor_copy(s_all[:, 0, sl], mv[:, sl, 0])
                nc.vector.tensor_tensor(s_all[:, 1, sl], mv[:, sl, 0],
                                        mv[:, sl, 0], Alu.mult)
                nc.vector.tensor_tensor(s_all[:, 1, sl], s_all[:, 1, sl],
                                        mv[:, sl, 1], Alu.add)

            def gn_stats_act(b, xts, xb_sb, ss, scr, t):
                # ACT path: Identity pass doubles as xb = x + bpp; Square pass
                # into a throwaway fp8 scratch.  ss[:, 0/1, t-2] = row sums.
                i = t - 2
                nc.scalar.activation(xb_sb[:, t, :], xts[t][:], Act.Identity,
                                     bias=vec_sb["bpp"][:, t:t + 1],
                                     accum_out=ss[:, 0, i:i + 1])
                nc.scalar.activation(scr[:], xts[t][:], Act.Square,
                                     accum_out=ss[:, 1, i:i + 1])

            def gn_ss_to_sall(b, ss, s_all, t):
                # accum(Identity+bpp) = sum(x) + N*bpp  ->  mean = acc/N - bpp
                i = t - 2
                nc.vector.tensor_scalar(s_all[:, 0, t:t + 1], ss[:, 0, i:i + 1],
                                        1.0 / N, vec_sb["bpp"][:, t:t + 1],
                                        Alu.mult, Alu.subtract)
                nc.vector.tensor_scalar(s_all[:, 1, t:t + 1], ss[:, 1, i:i + 1],
                                        1.0 / N, None, Alu.mult)

            def gn_tail_pair(b, s_all, ab, tp):
                # group-combine + rsqrt for tiles (2tp, 2tp+1) -> ab[:,0/1,t]
                t0 = 2 * tp
                sl = slice(t0, t0 + 2)
                gs = pssm.tile([128, 2, 2], F32, tag="sm", name=f"gs{b}_{tp}")
                nc.tensor.matmul(gs[:], sel_sb, s_all[:, :, sl],
                                 start=True, stop=True)
                gsb = smallp.tile([128, 2, 2], F32, tag="gsb",
                                  name=f"gb{b}_{tp}")
                nc.vector.tensor_copy(gsb[:], gs[:])
                sc = smallp.tile([128, 2, 2], F32, tag="sc", name=f"sc{b}_{tp}")
                va = sc[:, 0, :]
                tmp = sc[:, 1, :]
                y = ab[:, 0, sl]
                nc.vector.tensor_tensor(va, gsb[:, 0, :], gsb[:, 0, :], Alu.mult)
                nc.vector.tensor_tensor(va, gsb[:, 1, :], va, Alu.subtract)
                nc.vector.tensor_scalar_add(va, va, EPS)
                # rstd = rsqrt(var+eps) on DVE (no ACT table swap)
                nc.vector.tensor_scalar(y.bitcast(I32), va.bitcast(I32), 1,
                                        None, Alu.arith_shift_right)
                nc.vector.tensor_tensor(y.bitcast(I32),
                                        magic_sb[:].to_broadcast([128, 2]),
                                        y.bitcast(I32), Alu.subtract)
                for _ in range(2):  # Newton: y *= 1.5 - 0.5*v*y^2
                    nc.vector.tensor_tensor(tmp, y, y, Alu.mult)
                    nc.vector.tensor_tensor(tmp, tmp, va, Alu.mult)
                    nc.vector.tensor_scalar(tmp, tmp, -0.5, 1.5, Alu.mult,
                                            Alu.add)
                    nc.vector.tensor_tensor(y, y, tmp, Alu.mult)
                # a = rstd*gamma (in place over y); b = beta - mean_g*a
                nc.vector.tensor_tensor(y, y, vec_sb["gamma"][:, sl], Alu.mult)
                bsh = ab[:, 1, sl]
                nc.vector.tensor_tensor(bsh, gsb[:, 0, :], y, Alu.mult)
                nc.vector.tensor_tensor(bsh, vec_sb["beta"][:, sl], bsh,
                                        Alu.subtract)

            def hn_evict(b, xts, ab, hn_sb, t, on_act):
                if on_act:
                    nc.scalar.activation(hn_sb[:, t, :], xts[t][:],
                                         Act.Identity,
                                         bias=ab[:, 1, t:t + 1],
                                         scale=ab[:, 0, t:t + 1])
                else:
                    nc.vector.tensor_scalar(hn_sb[:, t, :], xts[t][:],
                                            ab[:, 0, t:t + 1],
                                            ab[:, 1, t:t + 1],
                                            Alu.mult, Alu.add)

            def mk_ps2(nm):
                return [psq.tile([128, 512], F32, tag="mm", name=f"{nm}_{ch}")
                        for ch in range(NCH)]

            def qk_pass(b, hn_sb, ps2, wname, t, itp):
                lhs = w_sb[wname][:, 2 * itp:2 * itp + 2, t * 128:(t + 1) * 128]
                for ch in range(NCH):
                    cs = slice(ch * 512, (ch + 1) * 512)
                    nc.tensor.matmul(ps2[ch][:], lhs,
                                     hn_sb[:, 2 * itp:2 * itp + 2, cs],
                                     start=(itp == 0), stop=(itp == 1),
                                     perf_mode=DR)

            def qk_evict(b, ps2, dst, bname, t, on_act):
                bias = vec_sb[bname][:, t:t + 1]
                for ch in range(NCH):
                    cs = slice(ch * 512, (ch + 1) * 512)
                    if on_act:
                        nc.scalar.activation(dst[:, t, cs], ps2[ch][:],
                                             Act.Identity, bias=bias,
                                             scale=1.0 / WS)
                    else:
                        nc.vector.tensor_scalar(dst[:, t, cs], ps2[ch][:],
                                                1.0 / WS, bias,
                                                Alu.mult, Alu.add)

            def qk_tile(b, hn_sb, dst, wname, bname, t, on_act):
                ps2 = mk_ps2(f"ps_{wname}{b}_{t}")
                qk_pass(b, hn_sb, ps2, wname, t, 0)
                qk_pass(b, hn_sb, ps2, wname, t, 1)
                qk_evict(b, ps2, dst, bname, t, on_act)

            def v_pass(b, hn_sb, ps, jt, itp):
                nc.tensor.matmul(
                    ps[:], hn_sb[:, 2 * itp:2 * itp + 2, jt * 128:(jt + 1) * 128],
                    w_sb["wv"][:, 2 * itp:2 * itp + 2, :],
                    start=(itp == 0), stop=(itp == 1), perf_mode=DR)

            def v_evict(b, ps, vT_sb, jt):
                nc.vector.tensor_scalar(vT_sb[:, jt, :], ps[:], 1.0 / WS,
                                        None, Alu.mult)

            def v_tile(b, hn_sb, vT_sb, jt):
                ps = psv.tile([128, 512], F32, tag="vmm", name=f"psv{b}_{jt}")
                v_pass(b, hn_sb, ps, jt, 0)
                v_pass(b, hn_sb, ps, jt, 1)
                v_evict(b, ps, vT_sb, jt)

            def s_tile(b, q_sb, k_sb, e_sb, jt):
                # e[:, jt, :] = exp(scale * k[:, :, jt-tile]^T @ q + EXPB)
                ps2 = mk_ps2(f"pss{b}_{jt}")
                for ctp in range(2):
                    lhs = k_sb[:, 2 * ctp:2 * ctp + 2, jt * 128:(jt + 1) * 128]
                    for ch in range(NCH):
                        cs = slice(ch * 512, (ch + 1) * 512)
                        nc.tensor.matmul(ps2[ch][:], lhs,
                                         q_sb[:, 2 * ctp:2 * ctp + 2, cs],
                                         start=(ctp == 0), stop=(ctp == 1),
                                         perf_mode=DR)
                for ch in range(NCH):
                    cs = slice(ch * 512, (ch + 1) * 512)
                    nc.scalar.activation(e_sb[:, jt, cs], ps2[ch][:], Act.Exp,
                                         bias=expb_sb[:], scale=SCALE)

            def dsum_make(b):
                # [2, 512]: DR ldweights requires M >= 2; both rows get the sum
                return [pssm.tile([2, 512], F32, tag="sm", name=f"d{b}_{ch}")
                        for ch in range(NCH)]

            def dsum_dr(b, psd, e_sb, r):
                # DR round r reduces j-tiles (2r, 2r+1) into the [1,512] rows
                for ch in range(NCH):
                    cs = slice(ch * 512, (ch + 1) * 512)
                    nc.tensor.matmul(psd[ch][:], ones_sb[:, :, 0:2],
                                     e_sb[:, 2 * r:2 * r + 2, cs],
                                     start=(r == 0), stop=(r == 3),
                                     perf_mode=DR)

            def dsum_tail(b, psd):
                drow = rp.tile([1, N], BF16, tag="drow", name=f"dr{b}")
                recip_sb = rp.tile([128, N], F32, tag="recip", name=f"rc{b}")
                for ch in range(NCH):
                    cs = slice(ch * 512, (ch + 1) * 512)
                    nc.vector.tensor_copy(drow[:, cs], psd[ch][0:1, :])
                for ch in range(NCH):
                    cs = slice(ch * 512, (ch + 1) * 512)
                    bc = pssm.tile([128, 512], F32, tag="sm", name=f"bc{b}_{ch}")
                    nc.tensor.matmul(bc[:], onescol_sb[:], drow[:, cs],
                                     start=True, stop=True)
                    nc.vector.reciprocal_approx_fast(recip_sb[:, cs], bc[:])
                return recip_sb

            def pv_tile(b, vT_sb, e_sb, recip_sb, ou_sb, ct):
                ps2 = mk_ps2(f"pso{b}_{ct}")
                for jtp in range(4):
                    lhs = vT_sb[:, 2 * jtp:2 * jtp + 2, ct * 128:(ct + 1) * 128]
                    for ch in range(NCH):
                        cs = slice(ch * 512, (ch + 1) * 512)
                        nc.tensor.matmul(ps2[ch][:], lhs,
                                         e_sb[:, 2 * jtp:2 * jtp + 2, cs],
                                         start=(jtp == 0), stop=(jtp == 3),
                                         perf_mode=DR)
                for ch in range(NCH):
                    cs = slice(ch * 512, (ch + 1) * 512)
                    nc.vector.tensor_tensor(ou_sb[:, ct, cs], ps2[ch][:],
                                            recip_sb[:, cs], Alu.mult)

            def xb_tile(b, xts, xb_sb, t):
                nc.scalar.activation(xb_sb[:, t, :], xts[t][:], Act.Identity,
                                     bias=vec_sb["bpp"][:, t:t + 1])

            def proj_tile(b, ou_sb, xb_sb, ot, out_engs, split_dma=False):
                ps2 = mk_ps2(f"psp{b}_{ot}")
                for ctp in range(2):
                    lhs = w_sb["wp"][:, 2 * ctp:2 * ctp + 2,
                                     ot * 128:(ot + 1) * 128]
                    for ch in range(NCH):
                        cs = slice(ch * 512, (ch + 1) * 512)
                        nc.tensor.matmul(ps2[ch][:], lhs,
                                         ou_sb[:, 2 * ctp:2 * ctp + 2, cs],
                                         start=(ctp == 0), stop=(ctp == 1),
                                         perf_mode=DR)
                o_sb = outp.tile([128, N], BF16, tag="o", name=f"o{b}_{ot}")
                for ch in range(NCH):
                    cs = slice(ch * 512, (ch + 1) * 512)
                    nc.vector.scalar_tensor_tensor(o_sb[:, cs], ps2[ch][:],
                                                   1.0 / (WS * OUS),
                                                   xb_sb[:, ot, cs],
                                                   Alu.mult, Alu.add)
                    if split_dma:
                        out_engs[ch].dma_start(out_ext[b, :, ot, cs],
                                               o_sb[:, cs])
                if not split_dma:
                    out_engs[0].dma_start(out_ext[b, :, ot, :], o_sb[:])

            # ================= schedule =================
            # ---- head: item0 GroupNorm, engine-split stats ----
            mv0 = smallp.tile([128, CT, 2], F32, tag="mv", name="mv0")
            s_all0 = smallp.tile([128, 2, CT], F32, tag="s_all", name="sa0")
            ab0 = smallp.tile([128, 2, CT], F32, tag="ab", name="ab0")
            ss0 = smallp.tile([128, 2, 2], F32, tag="ss", name="ss0")
            xb0 = xbp.tile([128, CT, N], BF16, tag="xb", name="xb0")
            scr0 = scrp.tile([128, N], F8, tag="scr", name="scr0")

            # ACT-side stats for tiles 2,3 (queue order: these run first)
            gn_stats_act(0, x0, xb0, ss0, scr0, 2)
            gn_stats_act(0, x0, xb0, ss0, scr0, 3)

            warm_burst(12)
            gn_stats_dve(0, x0, mv0, 0)
            warm_poke(mv0[:, 0, :])
            warm_burst(3)
            gn_stats_dve(0, x0, mv0, 1)
            warm_poke(mv0[:, 1, :])
            warm_burst(3)
            gn_mv_to_sall(0, mv0, s_all0, 0)
            gn_tail_pair(0, s_all0, ab0, 0)
            warm_poke(ab0[:, 0, 0:2])
            warm_burst(4, stop=True)
            hn0 = hnp.tile([128, CT, N], F8, tag="hn", name="hn0")
            hn_evict(0, x0, ab0, hn0, 0, on_act=False)
            hn_evict(0, x0, ab0, hn0, 1, on_act=False)
            gn_ss_to_sall(0, ss0, s_all0, 2)
            gn_ss_to_sall(0, ss0, s_all0, 3)
            gn_tail_pair(0, s_all0, ab0, 1)
            hn_evict(0, x0, ab0, hn0, 2, on_act=True)
            hn_evict(0, x0, ab0, hn0, 3, on_act=False)

            # ---- phase 1: QKV(0) with early first-passes + GN(1) ----
            q0 = qkp.tile([128, CT, N], F8, tag="q", name="q0")
            k0 = qkp.tile([128, CT, N], F8, tag="k", name="k0")
            v0 = vp.tile([128, NT, 512], F8, tag="vT", name="vT0")

            # first DR passes need only hn tiles 0,1
            psK0 = mk_ps2("ps_wk0_0")
            qk_pass(0, hn0, psK0, "wk", 0, 0)
            psQ0 = mk_ps2("ps_wq0_0")
            qk_pass(0, hn0, psQ0, "wq", 0, 0)
            psV0 = psv.tile([128, 512], F32, tag="vmm", name="psv0_0")
            v_pass(0, hn0, psV0, 0, 0)
            psV1 = psv.tile([128, 512], F32, tag="vmm", name="psv0_1")
            v_pass(0, hn0, psV1, 1, 0)
            # second passes (wait on hn tiles 2,3)
            qk_pass(0, hn0, psK0, "wk", 0, 1)
            qk_evict(0, psK0, k0, "bk", 0, on_act=True)
            qk_pass(0, hn0, psQ0, "wq", 0, 1)
            qk_evict(0, psQ0, q0, "bq", 0, on_act=False)
            v_pass(0, hn0, psV0, 0, 1)
            v_evict(0, psV0, v0, 0)
            v_pass(0, hn0, psV1, 1, 1)
            v_evict(0, psV1, v0, 1)

            mv1 = smallp.tile([128, CT, 2], F32, tag="mv", name="mv1")
            s_all1 = smallp.tile([128, 2, CT], F32, tag="s_all", name="sa1")
            ab1 = smallp.tile([128, 2, CT], F32, tag="ab", name="ab1")
            ss1 = smallp.tile([128, 2, 2], F32, tag="ss", name="ss1")
            xb1 = xbp.tile([128, CT, N], BF16, tag="xb", name="xb1")
            scr1 = scrp.tile([128, N], F8, tag="scr", name="scr1")
            hn1 = hnp.tile([128, CT, N], F8, tag="hn", name="hn1")

            for t in range(1, CT):
                qk_tile(0, hn0, k0, "wk", "bk", t, on_act=True)
                qk_tile(0, hn0, q0, "wq", "bq", t, on_act=False)
                v_tile(0, hn0, v0, 2 * t)
                v_tile(0, hn0, v0, 2 * t + 1)
                if t == 1:
                    gn_stats_dve(1, x1, mv1, 0)
                    gn_stats_act(1, x1, xb1, ss1, scr1, 2)
                elif t == 2:
                    gn_stats_dve(1, x1, mv1, 1)
                    gn_mv_to_sall(1, mv1, s_all1, 0)
                    gn_tail_pair(1, s_all1, ab1, 0)
                    hn_evict(1, x1, ab1, hn1, 0, on_act=False)
                    gn_stats_act(1, x1, xb1, ss1, scr1, 3)
                else:
                    hn_evict(1, x1, ab1, hn1, 1, on_act=True)
                    gn_ss_to_sall(1, ss1, s_all1, 2)
                    gn_ss_to_sall(1, ss1, s_all1, 3)
                    gn_tail_pair(1, s_all1, ab1, 1)
                    hn_evict(1, x1, ab1, hn1, 2, on_act=True)
                    hn_evict(1, x1, ab1, hn1, 3, on_act=False)

            # ---- phase 2: S(0) + QKV(1) + lagged dsum(0) ----
            e0 = ep.tile([128, NT, N], F8, tag="e", name="e0")
            q1 = qkp.tile([128, CT, N], F8, tag="q", name="q1")
            k1 = qkp.tile([128, CT, N], F8, tag="k", name="k1")
            v1 = vp.tile([128, NT, 512], F8, tag="vT", name="vT1")
            psd0 = dsum_make(0)
            for r in range(CT):
                s_tile(0, q0, k0, e0, 2 * r)
                s_tile(0, q0, k0, e0, 2 * r + 1)
                qk_tile(1, hn1, k1, "wk", "bk", r, on_act=False)
                qk_tile(1, hn1, q1, "wq", "bq", r, on_act=False)
                if r < 3:
                    v_tile(1, hn1, v1, 2 * r)
                    v_tile(1, hn1, v1, 2 * r + 1)
                if r == 1:
                    xb_tile(1, x1, xb1, 0)
                if r == 2:
                    xb_tile(1, x1, xb1, 1)
                if r > 0:
                    dsum_dr(0, psd0, e0, r - 1)
            dsum_dr(0, psd0, e0, 3)
            r0 = dsum_tail(0, psd0)

            # ---- phase 3: PV(0) + proj(0) + S(1) + lagged dsum(1) ----
            ou0 = oup.tile([128, CT, N], F8, tag="ou", name="ou0")
            xb_tile(0, x0, xb0, 0)
            for ct in range(CT):
                pv_tile(0, v0, e0, r0, ou0, ct)
            xb_tile(0, x0, xb0, 1)

            e1 = ep.tile([128, NT, N], F8, tag="e", name="e1")
            psd1 = dsum_make(1)
            out_engs0 = [[nc.sync], [nc.gpsimd], [nc.sync], [nc.gpsimd]]
            for r in range(CT):
                s_tile(1, q1, k1, e1, 2 * r)
                s_tile(1, q1, k1, e1, 2 * r + 1)
                proj_tile(0, ou0, xb0, r, out_engs0[r])
                if r > 0:
                    dsum_dr(1, psd1, e1, r - 1)

            # ---- phase 4: deferred V(1) tail + dsum(1) + PV(1) + proj(1) ----
            v_tile(1, hn1, v1, 6)
            dsum_dr(1, psd1, e1, 3)
            v_tile(1, hn1, v1, 7)
            r1 = dsum_tail(1, psd1)
            ou1 = oup.tile([128, CT, N], F8, tag="ou", name="ou1")
            for ct in range(CT):
                pv_tile(1, v1, e1, r1, ou1, ct)
            for ot in range(CT):
                if ot < 3:
                    proj_tile(1, ou1, xb1, ot, out_engs0[ot])
                else:
                    proj_tile(1, ou1, xb1, ot, [nc.sync, nc.gpsimd],
                              split_dma=True)

    nc.compile()
    return nc


def _prep_vec(v):
    # [C] f32 -> [128, CT] with v_sb[p, t] = v[t*128 + p]
    return np.ascontiguousarray(
        np.asarray(v, dtype=np.float32).reshape(CT, 128).T)


def _prep_w(w):
    # [C, C] (out, in) -> lhsT layout [128, CT, 512] fp8e4, pre-scaled by WS:
    # w_sb[p, it, o] = w[o, it*128 + p] * WS
    wT = np.asarray(w, dtype=np.float32).T * WS
    arr = wT.reshape(CT, 128, C).transpose(1, 0, 2)
    return np.clip(np.ascontiguousarray(arr), -240.0, 240.0).astype(
        ml_dtypes.float8_e4m3)


def kernel(x, gamma, beta, wq, bq, wk, bk, wv, bv, wp, bp):
    from concourse.bass_utils import run_bass_kernel_spmd

    nc = _CACHE.get("nc")
    if nc is None:
        nc = _CACHE["nc"] = _build_bass()

    x = np.asarray(x, dtype=np.float32)
    # [16, C, H, W] -> [16, 128, CT, N] bf16
    xr = np.ascontiguousarray(
        x.reshape(B_TOT, CT, 128, N).transpose(0, 2, 1, 3)).astype(
        ml_dtypes.bfloat16)

    bpp = np.asarray(wp, np.float32) @ np.asarray(bv, np.float32) \
        + np.asarray(bp, np.float32)
    sel = np.kron(np.eye(128 // GS, dtype=np.float32),
                  np.full((GS, GS), 1.0 / GS, dtype=np.float32))
    cb = np.empty((128, CB_W), dtype=np.float32)
    for i, v in enumerate((gamma, beta, bq, bk, bpp)):
        cb[:, i * CT:(i + 1) * CT] = _prep_vec(v)
    cb[:, NVEC * CT:] = sel
    common = {
        "wq": _prep_w(wq), "wk": _prep_w(wk), "wv": _prep_w(wv),
        "wp": _prep_w(wp), "cb": cb,
    }
    in_maps = [
        {"x": np.ascontiguousarray(xr[c * BPC:(c + 1) * BPC]), **common}
        for c in range(NCORES)
    ]
    res = run_bass_kernel_spmd(nc, in_maps, core_ids=list(range(NCORES)))
    # [BPC, 128, CT, N] bf16 per core -> [16, C, H, W] f32
    out = np.concatenate([np.asarray(r["out"]) for r in res.results], axis=0)
    out = out.astype(np.float32)
    return np.ascontiguousarray(
        out.transpose(0, 2, 1, 3)).reshape(B_TOT, C, H, W)
